# revision 2
# baseline (speedup 1.0000x reference)
"""Selective SSM (Mamba-1 style) layer on 8 Trainium2 NeuronCores — v2.

Sharding: core c -> batch b = c // 2, d_model half dh = c % 2 (512 channels).
Cores fully independent (recurrence elementwise in d), no collectives.

v2 layout [d on partitions, t on free]:
  - B/C projections first -> DRAM bounce -> half-0 broadcast DMAs prefetch
    under the delta projection (PE) + Softplus (ACT, single op - no table
    thrash).
  - Scan phase: halves of 8 states, m-inner; yps accumulates all 8 states
    of a half in PSUM; half-0 folds via ACT copy, half-1 via DVE add.
  - bar = exp(A*dt) in bf16 (validated: absmax rel 1.33e-2 vs 1.36e-2 f32).
  - prod muls for a subset of states offloaded to GpSimd (Pool) to shave
    the DVE wall (scan is DVE-only: 2 cycles/elem, no fast mode).
  - Tail: per t-tile, 4 transposes into one PSUM bank + DVE add of the
    prescaled skip (x*D), store.
"""

import numpy as np
import ml_dtypes
from contextlib import ExitStack

import concourse.bacc as bacc
import concourse.bass as bass
import concourse.mybir as mybir
import concourse.tile as tile
from concourse.bass_utils import run_bass_kernel_spmd

BF16 = ml_dtypes.bfloat16
F32 = mybir.dt.float32
B16 = mybir.dt.bfloat16

B_SZ, SEQ, D, N = 4, 2048, 1024, 16
DL = 512            # d_model channels per core
ND = DL // 128      # 4 d-tiles
NK = D // 128       # 8 contraction tiles
TB = SEQ // 512     # 4 moving-dim blocks for matmul
NT = SEQ // 128     # 16 t-tiles for output
NHALF = 2
NH = N // NHALF     # 8 states per half

# GpSimd offload disabled: Pool TT runs ~4.8us/tile AND its SBUF port
# contention inflates concurrent DVE TT muls from 1.2us to 5.1us (measured)
POOL_PROD_JS = ()
POOL_U_JS = ()

_CACHE = {}


def _build():
    if "nc" in _CACHE:
        return _CACHE["nc"]
    mult = mybir.AluOpType.mult
    add = mybir.AluOpType.add

    nc = bacc.Bacc("TRN2", target_bir_lowering=False, debug=False, num_devices=8)

    xb16_d = nc.dram_tensor("xb16", [SEQ, D], B16, kind="ExternalInput")
    # (x * D_skip)^T for this core's channels, host-transposed [d, t]
    xdt16_d = nc.dram_tensor("xdt16", [DL, SEQ], B16, kind="ExternalInput")
    wd16_d = nc.dram_tensor("wd16", [D, DL], B16, kind="ExternalInput")
    wb16_d = nc.dram_tensor("wb16", [D, N], B16, kind="ExternalInput")
    wc16_d = nc.dram_tensor("wc16", [D, N], B16, kind="ExternalInput")
    aneg_d = nc.dram_tensor("aneg", [DL, N], F32, kind="ExternalInput")
    bd_d = nc.dram_tensor("bd", [DL, 1], F32, kind="ExternalInput")
    bb_d = nc.dram_tensor("bb", [N, 1], F32, kind="ExternalInput")
    bc_d = nc.dram_tensor("bc", [N, 1], F32, kind="ExternalInput")
    id16_d = nc.dram_tensor("id16", [128, 128], B16, kind="ExternalInput")
    id32_d = nc.dram_tensor("id32", [128, 128], F32, kind="ExternalInput")
    y_d = nc.dram_tensor("y", [SEQ, DL], F32, kind="ExternalOutput")

    with tile.TileContext(nc) as tc, ExitStack() as ctx:
        consts = ctx.enter_context(tc.tile_pool(name="consts", bufs=1))
        persist = ctx.enter_context(tc.tile_pool(name="persist", bufs=1))
        big = ctx.enter_context(tc.tile_pool(name="big", bufs=1))
        ps = ctx.enter_context(tc.tile_pool(name="ps", bufs=2, space="PSUM"))
        work = ctx.enter_context(tc.tile_pool(name="work", bufs=2))
        dram = ctx.enter_context(tc.tile_pool(name="dram", bufs=1, space="DRAM"))

        # ---- B/C weights first (tiny, gate the B/C projection), then the
        # x^T transposes, then the rest of the constants ----
        # per k: [Wb_k | Wc_k] interleaved so one matmul computes both projs
        wbcall = consts.tile([128, 2 * NK * N], B16, tag="wbcall", name="wbcall")
        for k in range(NK):
            nc.sync.dma_start(wbcall[:, k * 2 * N:k * 2 * N + N],
                              wb16_d[k * 128:(k + 1) * 128, :])
            nc.sync.dma_start(wbcall[:, k * 2 * N + N:(k + 1) * 2 * N],
                              wc16_d[k * 128:(k + 1) * 128, :])
        wbc_sb = [wbcall[:, k * 2 * N:(k + 1) * 2 * N] for k in range(NK)]

        xt = []
        for k in range(NK):
            t = big.tile([128, SEQ], B16, tag="xt", bufs=NK, name=f"xt{k}")
            nc.sync.dma_start_transpose(t[:], xb16_d[:, k * 128:(k + 1) * 128])
            xt.append(t)

        wdall = consts.tile([128, NK * DL], B16, tag="wdall", name="wdall")
        for k in range(NK):
            nc.sync.dma_start(wdall[:, k * DL:(k + 1) * DL],
                              wd16_d[k * 128:(k + 1) * 128, :])
        wd_sb = [wdall[:, k * DL:(k + 1) * DL] for k in range(NK)]
        # abdall: per m, 17 cols: A[:,0..15], bd
        abdall = consts.tile([128, ND * 17 + 2], F32, tag="abd", name="abdall")
        for m in range(ND):
            nc.sync.dma_start(abdall[:, m * 17:m * 17 + 16],
                              aneg_d[m * 128:(m + 1) * 128, :])
            nc.sync.dma_start(abdall[:, m * 17 + 16:m * 17 + 17],
                              bd_d[m * 128:(m + 1) * 128, :])
        a_col = lambda m, n: abdall[:, m * 17 + n:m * 17 + n + 1]
        bd_col = lambda m: abdall[:, m * 17 + 16:m * 17 + 17]
        # bb on partitions 0..15, bc on 16..31 of one column (joint bias AP)
        nc.sync.dma_start(abdall[0:N, ND * 17:ND * 17 + 1], bb_d[:, :])
        nc.sync.dma_start(abdall[N:2 * N, ND * 17:ND * 17 + 1], bc_d[:, :])
        bbc_sb = abdall[0:2 * N, ND * 17:ND * 17 + 1]
        id16_sb = consts.tile([128, 128], B16, tag="id16", name="id16sb")
        nc.sync.dma_start(id16_sb[:], id16_d[:, :])
        id32_sb = consts.tile([128, 128], F32, tag="id32", name="id32sb")
        nc.sync.dma_start(id32_sb[:], id32_d[:, :])

        # ---- B+C projection in one matmul pass -> bcall [32, SEQ] bf16 ----
        # (bcall borrows a bc-ring slot; it is dead after the DRAM bounce)
        bcall = big.tile([128, SEQ], B16, tag="bc", bufs=16, name="bcall")
        psb = ps.tile([128, SEQ], F32, tag="yps", name="psb")
        for tb in range(TB):
            blk = slice(tb * 512, (tb + 1) * 512)
            for k in range(NK):
                nc.tensor.matmul(psb[0:2 * N, blk], wbc_sb[k], xt[k][:, blk],
                                 start=(k == 0), stop=(k == NK - 1))
            nc.scalar.activation(bcall[0:2 * N, blk], psb[0:2 * N, blk],
                                 mybir.ActivationFunctionType.Identity,
                                 bias=bbc_sb, scale=1.0)
        bcall_dr = dram.tile([2 * N, SEQ], B16, tag="bcall_dr", name="bcall_dr")
        nc.sync.dma_start(bcall_dr[:], bcall[0:2 * N, :])

        # ---- prefetch half-0 B/C partition broadcasts (overlap delta proj) --
        def bcast_pair(n):
            br = big.tile([128, SEQ], B16, tag="bc", bufs=16, name=f"brep{n}")
            nc.sync.dma_start(br[:], bcall_dr[n:n + 1, :].partition_broadcast(128))
            cr = big.tile([128, SEQ], B16, tag="bc", bufs=16, name=f"crep{n}")
            nc.sync.dma_start(cr[:], bcall_dr[N + n:N + n + 1, :].partition_broadcast(128))
            return br, cr

        bc_tiles = {}
        for j in range(NH):
            bc_tiles[j] = bcast_pair(j)

        # ---- delta projection + Softplus -> dt (packed bf16) ----
        dtall = persist.tile([128, ND * SEQ], B16, tag="dt", name="dtall")
        dt_m = [dtall[:, m * SEQ:(m + 1) * SEQ] for m in range(ND)]
        # softplus = Ln(1 + Exp(z + bd)), Exp per tb block (PSUM -> dtall
        # bf16), then one full-width Ln in place. The Exp<->Ln table loads
        # (2 per m) hide in ACT's slack under the delta-proj matmuls; per-m
        # grouping keeps dt[0] early so the first scan isn't gated on m=3.
        for m in range(ND):
            psd = ps.tile([128, SEQ], F32, tag="yps", name="psd")
            for tb in range(TB):
                blk = slice(tb * 512, (tb + 1) * 512)
                for k in range(NK):
                    nc.tensor.matmul(
                        psd[:, blk], wd_sb[k][:, m * 128:(m + 1) * 128],
                        xt[k][:, blk], start=(k == 0), stop=(k == NK - 1))
            for tb in range(TB):
                blk = slice(tb * 512, (tb + 1) * 512)
                nc.scalar.activation(
                    dt_m[m][:, blk], psd[:, blk],
                    mybir.ActivationFunctionType.Exp,
                    bias=bd_col(m), scale=1.0)
            nc.scalar.activation(
                dt_m[m][:], dt_m[m][:],
                mybir.ActivationFunctionType.Ln, bias=1.0, scale=1.0)

        # ---- scan phase: halves, m-inner ----
        y_sb = [persist.tile([128, SEQ], F32, tag=f"ysb{m}", name=f"ysb{m}")
                for m in range(ND)]

        def emit_tail(m):
            # transpose y[m] to [t, d] strips and store; runs under the next
            # m's scan window
            pst = ps.tile([128, SEQ], F32, tag="yps", name="pst")
            for tt in range(NT):
                nc.tensor.transpose(
                    pst[:, tt * 128:(tt + 1) * 128],
                    y_sb[m][:, tt * 128:(tt + 1) * 128], id32_sb[:])
            for q in range(4):
                qblk = slice(q * 512, (q + 1) * 512)
                yout = work.tile([128, DL], F32, tag="yout", bufs=2,
                                 name="yout")
                nc.scalar.activation(yout[:], pst[:, qblk],
                                     mybir.ActivationFunctionType.Copy,
                                     bias=0.0, scale=1.0)
                for s in range(4):
                    tt = q * 4 + s
                    nc.sync.dma_start(
                        y_d[tt * 128:(tt + 1) * 128, m * 128:(m + 1) * 128],
                        yout[:, s * 128:(s + 1) * 128])

        pending_tail = [None]
        for half in range(NHALF):
            if half == 1:
                for j in range(NH):
                    bc_tiles[j] = bcast_pair(NH + j)
            for m in range(ND):
                # xb16 is column-permuted per core so this core's own d-half
                # occupies xt[0..3]; weights are row-permuted to match.
                dtx = work.tile([128, SEQ], B16, tag="dtx", name="dtx")
                nc.vector.tensor_mul(dtx[:], dt_m[m][:], xt[m][:])
                yps = ps.tile([128, SEQ], F32, tag="yps", name="yps")
                if half == 1:
                    # inject the half-0 partial on PE so no DVE fold is needed
                    for tb in range(TB):
                        blk = slice(tb * 512, (tb + 1) * 512)
                        nc.tensor.matmul(yps[:, blk], id32_sb[:],
                                         y_sb[m][:, blk], start=True, stop=False)
                for j in range(NH):
                    n = half * NH + j
                    brep, crep = bc_tiles[j]
                    bar = work.tile([128, SEQ], B16, tag="bar", bufs=3, name="bar")
                    nc.scalar.activation(
                        bar[:], dt_m[m][:], mybir.ActivationFunctionType.Exp,
                        bias=0.0, scale=a_col(m, n))
                    u = work.tile([128, SEQ], B16, tag="u", name="u")
                    if j in POOL_U_JS:
                        nc.gpsimd.tensor_mul(u[:], dtx[:], brep[:])
                    else:
                        nc.vector.tensor_mul(u[:], dtx[:], brep[:])
                    h = work.tile([128, SEQ], B16, tag="h", name="h")
                    nc.vector.tensor_tensor_scan(
                        h[:], bar[:], u[:], 0.0, op0=mult, op1=add)
                    prod = work.tile([128, SEQ], B16, tag="prod", name="prod")
                    if j in POOL_PROD_JS:
                        nc.gpsimd.tensor_mul(prod[:], h[:], crep[:])
                    else:
                        nc.vector.tensor_mul(prod[:], h[:], crep[:])
                    last = (half == 1 and j == NH - 1)
                    for tb in range(TB):
                        blk = slice(tb * 512, (tb + 1) * 512)
                        nc.tensor.matmul(yps[:, blk], id16_sb[:], prod[:, blk],
                                         start=(half == 0 and j == 0),
                                         stop=last)
                    if j == 0 and pending_tail[0] is not None:
                        emit_tail(pending_tail[0])
                        pending_tail[0] = None
                if half == 0:
                    # fold the prescaled skip (x*D)^T into the PSUM sum on PE
                    xdts = work.tile([128, SEQ], B16, tag="xdts", bufs=1,
                                     name="xdts")
                    nc.sync.dma_start(xdts[:], xdt16_d[m * 128:(m + 1) * 128, :])
                    for tb in range(TB):
                        blk = slice(tb * 512, (tb + 1) * 512)
                        nc.tensor.matmul(yps[:, blk], id16_sb[:], xdts[:, blk],
                                         start=False, stop=True)
                    nc.scalar.activation(
                        y_sb[m][:], yps[:], mybir.ActivationFunctionType.Copy,
                        bias=0.0, scale=1.0)
                else:
                    nc.scalar.activation(
                        y_sb[m][:], yps[:], mybir.ActivationFunctionType.Copy,
                        bias=0.0, scale=1.0)
                    pending_tail[0] = m
        if pending_tail[0] is not None:
            emit_tail(pending_tail[0])

    nc.compile()
    _CACHE["nc"] = nc
    return nc


def _in_maps(x, A_log, D_skip, Wd, bd, Wb, bb, Wc, bc):
    A = (-np.exp(np.asarray(A_log, np.float64))).astype(np.float32)
    x = np.asarray(x, np.float32)
    maps = []
    for c in range(8):
        b, dh = c // 2, c % 2
        dsl = slice(dh * DL, (dh + 1) * DL)
        # own d-half first, so the kernel's xt[0..3] are this core's channels
        perm = np.r_[dh * DL:(dh + 1) * DL, (1 - dh) * DL:(2 - dh) * DL]
        maps.append({
            "xb16": x[b][:, perm].astype(BF16),
            "xdt16": np.ascontiguousarray(
                (x[b][:, dsl] * np.asarray(D_skip)[None, dsl]).T).astype(BF16),
            "wd16": np.asarray(Wd)[perm][:, dsl].astype(BF16),
            "wb16": np.asarray(Wb)[perm].astype(BF16),
            "wc16": np.asarray(Wc)[perm].astype(BF16),
            "aneg": A[dsl],
            "bd": np.asarray(bd, np.float32)[dsl].reshape(DL, 1),
            "bb": np.asarray(bb, np.float32).reshape(N, 1),
            "bc": np.asarray(bc, np.float32).reshape(N, 1),
            "id16": np.eye(128, dtype=BF16),
            "id32": np.eye(128, dtype=np.float32),
        })
    return maps


def kernel(x, A_log, D_skip, Wd, bd, Wb, bb, Wc, bc, _trace=False):
    nc = _build()
    maps = _in_maps(x, A_log, D_skip, Wd, bd, Wb, bb, Wc, bc)
    res = run_bass_kernel_spmd(nc, maps, list(range(8)), trace=_trace)
    y = np.zeros((B_SZ, SEQ, D), np.float32)
    for c, om in enumerate(res.results):
        b, dh = c // 2, c % 2
        y[b][:, dh * DL:(dh + 1) * DL] = om["y"]
    if _trace:
        kernel.last_result = res
    return y
